# revision 2
# baseline (speedup 1.0000x reference)
"""DynamicGCN (3-layer GCN + temporal gate) on 8 trn2 NeuronCores via Bass.

v2: feature-major edge aggregation directly in PSUM.

Nodes are partitioned contiguously across the 8 cores (12544 rows each,
padded). Per layer:

  1. h' = dinv * (x @ W) for own nodes (PE matmul node-major, ACT eviction
     applies dinv), kept in SBUF (h_sb) and staged per src-quarter to DRAM.
  2. Four chunked AllGathers (one per src-quarter) build int16-indexable
     gather tables (<= 25600 rows each).
  3. Per quarter ("round"), edges sorted by dst are packed into fixed
     dst-windows: tile t covers dst in [W*t, W*t+512). dma_gather pulls the
     128 src rows of each tile; a DVE tensor_scalar generates the indicator
     rhs on the fly: ind[e, j] = (iota[j] == dstrel[e]) * dinv[dst[e]].
     One or two PE matmuls per tile accumulate msg^T @ ind into the
     feature-major PSUM tile of the 512-node chunk(s) the window covers.
  4. The self-loop term is injected by diagonal init matmuls (start=True);
     between rounds partial sums bounce through SBUF (pacc) via identity
     reload matmuls. After round 3: relu (+bias) * gate evicts straight
     into xT (already transposed for the next layer's matmul). The last
     layer transposes back to node-major and DMAs to the output.

The temporal gate MLP runs once on-device at kernel start.
"""
import sys, os, types

for _p in ("/opt/trn_rl_repo", os.path.dirname(os.path.abspath(__file__))):
    if _p not in sys.path:
        sys.path.insert(0, _p)

import numpy as np

# ---------------------------------------------------------------- shims ----
def _install_shims():
    # NTFF profile hook (missing module in this container; used for trace=True)
    if "antenv.axon_hooks" not in sys.modules:
        try:
            import antenv
            from trn_agent_boot.trn_boot import _ntff_profile_via_ctypes

            mod = types.ModuleType("antenv.axon_hooks")
            _state = {"hook": None}
            mod.set_axon_ntff_profile_hook = lambda h: _state.__setitem__("hook", h)
            mod.get_axon_ntff_profile_hook = lambda: _state["hook"]
            sys.modules["antenv.axon_hooks"] = mod
            antenv.axon_hooks = mod
            if os.path.exists("/opt/axon/libaxon_pjrt.so"):
                mod.set_axon_ntff_profile_hook(
                    _ntff_profile_via_ctypes("/opt/axon/libaxon_pjrt.so")
                )
        except Exception:
            pass

    # walrus in this container rejects >1 sync wait per instruction; split
    # extra waits onto same-engine NoOps (identical semantics).
    import concourse.bass as bass
    import orjson

    if getattr(bass.Bass.to_json_bytes, "_waitsplit", False):
        return

    orig = bass.Bass.to_json_bytes

    def _split(j):
        ctr = 0
        for fn in j.get("functions", []):
            for bb in fn.get("blocks", []):
                out, changed = [], False
                for ins in bb.get("instructions", []):
                    si = ins.get("sync_info")
                    waits = (si or {}).get("on_wait") or []
                    if len(waits) > 1 and ins.get("engine") not in (None, "Unassigned"):
                        for w in waits[:-1]:
                            ctr += 1
                            out.append({
                                "debug": ins.get("debug", 0), "engine": ins["engine"],
                                "ins": [], "outs": [], "name": f"I-wsplit-{ctr}",
                                "opcode": "NoOp",
                                "sync_info": {"on_update": [], "on_wait": [w]},
                            })
                        si["on_wait"] = [waits[-1]]
                        changed = True
                    out.append(ins)
                if changed:
                    bb["instructions"] = out
        return j

    def to_json_bytes(self):
        return orjson.dumps(_split(orjson.loads(orig(self))))

    to_json_bytes._waitsplit = True
    bass.Bass.to_json_bytes = to_json_bytes


_install_shims()

import concourse.bass as bass
import concourse.mybir as mybir
import concourse.tile as tile
from concourse import library_config
from concourse.bass_utils import run_bass_kernel_spmd

f16 = mybir.dt.float16
f32 = mybir.dt.float32
i16 = mybir.dt.int16

# ---------------------------------------------------------- problem dims ---
N_NODES = 100000
N_EDGES = 600000
D = 128
N_LAYERS = 3
NC = 8
N_OWN = 12544                 # padded rows per core (= 98*128)
NQ = 4                        # src quarters (AG chunks / gather tables)
QSTART = [0, 3200, 6400, 9600, 12544]
QSIZE = [3200, 3200, 3200, 2944]
CHUNKS = 98                   # 128-row matmul chunks per core
QCHUNK = [0, 25, 50, 75, 98]  # 128-chunk boundaries of quarters
NCH = 25                      # 512-node psum chunks (24 full + 1 of 256)
CALL = 256                    # rows per dma_gather call
MAXSPAN = 640                 # max per-tile dst span (envelope over cores)


def _wrap_idx(vals):
    """int16 stream -> [128, n/16] tile layout (16-partition wrap, replicated
    for the 8 Q7 cores). vals length must be a multiple of 16."""
    a = np.asarray(vals, np.int16).reshape(-1, 16).T  # [16, n/16]
    return np.tile(a, (8, 1)).copy()


def _prep_graph(edge_index):
    """Partition/sort edges; strict 128-edge tiles with per-(q,t) static
    column spans (envelope over the 8 cores).

    Returns (dinv, gidx, dstrel, sdw, plan) where plan = per-quarter tuple of
    (a, b) node spans per tile; gidx/dstrel/sdw are [NC, sum_q T_q * 128].
    """
    src = np.asarray(edge_index[0], np.int64)
    dst = np.asarray(edge_index[1], np.int64)
    deg = np.bincount(dst, minlength=N_NODES).astype(np.float32) + 1.0
    dinv = (1.0 / np.sqrt(deg)).astype(np.float32)

    core = dst // N_OWN
    rank = src // N_OWN
    i_loc = src % N_OWN
    q = np.searchsorted(np.asarray(QSTART[1:4]), i_loc, side="right")
    qs = np.asarray([QSIZE[j] for j in range(NQ)])
    qst = np.asarray([QSTART[j] for j in range(NQ)])
    tabrow = rank * qs[q] + (i_loc - qst[q])
    dst_loc = dst % N_OWN
    sv = dinv[dst]

    # sorted per (core, quarter) streams
    streams = {}
    for c in range(NC):
        mc = core == c
        for qq in range(NQ):
            m = mc & (q == qq)
            order = np.argsort(dst_loc[m], kind="stable")
            streams[(c, qq)] = (tabrow[m][order], dst_loc[m][order],
                                sv[m][order])
    T_q = [max(-(-streams[(c, qq)][1].size // 128) for c in range(NC))
           for qq in range(NQ)]
    t_tot = sum(T_q)
    gidx = np.zeros((NC, t_tot * 128), np.int16)
    dstrel = np.full((NC, t_tot * 128), -1.0, np.float32)
    sdw = np.zeros((NC, t_tot * 128), np.float32)
    plan = []
    off = 0
    for qq in range(NQ):
        spans = []
        for t in range(T_q[qq]):
            a, b = N_OWN, 0
            for c in range(NC):
                dl = streams[(c, qq)][1]
                if t * 128 < dl.size:
                    a = min(a, int(dl[t * 128]))
                    b = max(b, int(dl[min(t * 128 + 127, dl.size - 1)]) + 1)
            if b <= a:
                spans.append((0, 0))
                continue
            assert b - a <= MAXSPAN, f"tile span {b - a} exceeds {MAXSPAN}"
            base = 512 * (a // 512)
            spans.append((a, b))
            for c in range(NC):
                tr, dl, svq = streams[(c, qq)]
                e0 = t * 128
                n = min(128, dl.size - e0)
                if n <= 0:
                    continue
                s = off + t * 128
                gidx[c, s:s + n] = tr[e0:e0 + n]
                dstrel[c, s:s + n] = dl[e0:e0 + n] - base
                sdw[c, s:s + n] = svq[e0:e0 + n]
        plan.append(tuple(spans))
        off += T_q[qq] * 128
    return dinv, gidx, dstrel, sdw, tuple(plan)


def _build(nc_prog, plan, has_bias):
    nc = nc_prog
    T_q = [len(spans) for spans in plan]
    t_tot = sum(T_q)
    # ---------------- I/O ----------------
    xT_in = nc.dram_tensor("xT_in", [128, N_OWN], f16, kind="ExternalInput")
    w_in = [nc.dram_tensor(f"w{l}", [128, 128], f16, kind="ExternalInput")
            for l in range(N_LAYERS)]
    dinv_in = nc.dram_tensor("dinv_in", [128, CHUNKS], f32, kind="ExternalInput")
    pidx_in = nc.dram_tensor("pidx_in", [128, 1], f32, kind="ExternalInput")
    iota_in = nc.dram_tensor("iota_in", [128, 2048], f16, kind="ExternalInput")
    gidx_in = nc.dram_tensor("gidx_in", [128, t_tot * 8], i16, kind="ExternalInput")
    dstrel_in = nc.dram_tensor("dstrel_in", [128, t_tot], f32, kind="ExternalInput")
    sdw_in = nc.dram_tensor("sdw_in", [128, t_tot], f32, kind="ExternalInput")
    ident_in = nc.dram_tensor("ident_in", [128, 128], f16, kind="ExternalInput")
    # gate MLP params
    wg1_in = nc.dram_tensor("wg1_in", [128, 1], f32, kind="ExternalInput")
    bg1_in = nc.dram_tensor("bg1_in", [128, 1], f32, kind="ExternalInput")
    wg2_in = nc.dram_tensor("wg2_in", [128, 128], f32, kind="ExternalInput")
    bg2_in = nc.dram_tensor("bg2_in", [128, 1], f32, kind="ExternalInput")
    ts_in = nc.dram_tensor("ts_in", [128, 1], f32, kind="ExternalInput")
    b_in = None
    if has_bias:
        b_in = [nc.dram_tensor(f"b{l}", [128, 1], f32, kind="ExternalInput")
                for l in range(N_LAYERS)]
    out_ext = nc.dram_tensor("out_ext", [N_OWN, 128], f32, kind="ExternalOutput")
    dbg = os.environ.get("KV2_DEBUG") == "1"
    if dbg:
        dbg_h = nc.dram_tensor("dbg_h", [128, CHUNKS * 128], f16,
                               kind="ExternalOutput")
        dbg_tab = nc.dram_tensor("dbg_tab", [NC * QSIZE[0], 128], f16,
                                 kind="ExternalOutput")
        dbg_x1 = nc.dram_tensor("dbg_x1", [128, N_OWN], f16,
                                kind="ExternalOutput")
        dbg_pacc = nc.dram_tensor("dbg_pacc", [128, N_OWN], f16,
                                  kind="ExternalOutput")

    # ---------------- internal DRAM ----------------
    hq = [nc.dram_tensor(f"hq{qq}", [QSIZE[qq], 128], f16) for qq in range(NQ)]
    tab = [nc.dram_tensor(f"tab{qq}", [NC * QSIZE[qq], 128], f16,
                          addr_space="Shared") for qq in range(NQ)]

    # per-quarter chunk touch maps (static, identical on all cores)
    chw = [512] * (NCH - 1) + [N_OWN - 512 * (NCH - 1)]  # last chunk 256
    first_t = []   # per q: {chunk: first tile}
    last_t = []    # per q: {chunk: last tile}
    for qq in range(NQ):
        ft, lt = {}, {}
        for t, (a, b) in enumerate(plan[qq]):
            if b <= a:
                continue
            for k in range(a // 512, (b - 1) // 512 + 1):
                if k not in ft:
                    ft[k] = t
                lt[k] = t
        first_t.append(ft)
        last_t.append(lt)

    with tile.TileContext(nc) as tc:
        with (
            tc.tile_pool(name="const", bufs=1) as cp,
            tc.tile_pool(name="msgp", bufs=24) as msgp,
            tc.tile_pool(name="indp", bufs=20) as indp,
            tc.tile_pool(name="diagp", bufs=8) as diagp,
            tc.tile_pool(name="evp", bufs=6) as evp,
            tc.tile_pool(name="psum_h", bufs=2, space="PSUM") as pp_h,
            tc.tile_pool(name="psum_agg", bufs=4, space="PSUM") as pp_agg,
            tc.tile_pool(name="psum_t", bufs=2, space="PSUM") as pp_t,
        ):
            nc.gpsimd.load_library(library_config.mlp)
            nregs = {CALL: nc.gpsimd.to_reg(CALL)}
            for qq in range(NQ):
                tn = T_q[qq] * 128 - (T_q[qq] * 128 // CALL) * CALL
                if tn and tn not in nregs:
                    nregs[tn] = nc.gpsimd.to_reg(tn)

            # persistent SBUF
            xT = cp.tile([128, N_OWN], f16)
            nc.sync.dma_start(out=xT[:], in_=xT_in[:])
            wt = []
            for l in range(N_LAYERS):
                w = cp.tile([128, 128], f16, tag=f"w{l}")
                nc.sync.dma_start(out=w[:], in_=w_in[l][:])
                wt.append(w)
            dinv_sb = cp.tile([128, CHUNKS], f32)
            nc.sync.dma_start(out=dinv_sb[:], in_=dinv_in[:])
            pidx = cp.tile([128, 1], f32)
            nc.sync.dma_start(out=pidx[:], in_=pidx_in[:])
            iota = cp.tile([128, 2048], f16)
            nc.sync.dma_start(out=iota[:], in_=iota_in[:])
            gidx = cp.tile([128, t_tot * 8], i16)
            nc.sync.dma_start(out=gidx[:], in_=gidx_in[:])
            dstrel = cp.tile([128, t_tot], f32)
            nc.sync.dma_start(out=dstrel[:], in_=dstrel_in[:])
            sdw = cp.tile([128, t_tot], f32)
            nc.sync.dma_start(out=sdw[:], in_=sdw_in[:])
            ident = cp.tile([128, 128], f16)
            nc.sync.dma_start(out=ident[:], in_=ident_in[:])
            h_sb = cp.tile([128, CHUNKS, 128], f16)
            pacc = cp.tile([128, N_OWN], f16)
            b_sb = []
            if has_bias:
                for l in range(N_LAYERS):
                    bt = cp.tile([128, 1], f32, tag=f"b{l}")
                    nc.sync.dma_start(out=bt[:], in_=b_in[l][:])
                    b_sb.append(bt)

            # ---------------- temporal gate ----------------
            wg1 = cp.tile([128, 1], f32)
            nc.sync.dma_start(out=wg1[:], in_=wg1_in[:])
            bg1 = cp.tile([128, 1], f32)
            nc.sync.dma_start(out=bg1[:], in_=bg1_in[:])
            wg2 = cp.tile([128, 128], f32)
            nc.sync.dma_start(out=wg2[:], in_=wg2_in[:])
            bg2 = cp.tile([128, 1], f32)
            nc.sync.dma_start(out=bg2[:], in_=bg2_in[:])
            tsr = cp.tile([128, 1], f32)
            nc.sync.dma_start(out=tsr[:], in_=ts_in[:])

            tmp1 = cp.tile([128, 1], f32, tag="g1")
            nc.vector.tensor_mul(tmp1[:], wg1[:], tsr[:])
            tanh1 = cp.tile([128, 1], f32, tag="g2")
            nc.scalar.activation(
                tanh1[:], tmp1[:], mybir.ActivationFunctionType.Tanh, bias=bg1[:])
            ps_g = pp_h.tile([128, 1], f32, tag="psh")
            nc.tensor.matmul(ps_g[:], lhsT=wg2[:], rhs=tanh1[:], start=True, stop=True)
            gate_col = cp.tile([128, 1], f32, tag="gcol")
            nc.scalar.activation(
                gate_col[:], ps_g[:], mybir.ActivationFunctionType.Sigmoid,
                bias=bg2[:])

            # ---------------- layers ----------------
            for l in range(N_LAYERS):
                last = l == N_LAYERS - 1
                # h' = dinv * (x @ W) into h_sb; DMA per quarter + AllGather
                for c in range(CHUNKS):
                    ps_h = pp_h.tile([128, 128], f32, tag="psh")
                    nc.tensor.matmul(
                        ps_h[:], lhsT=xT[:, c * 128:(c + 1) * 128],
                        rhs=wt[l][:], start=True, stop=True)
                    nc.scalar.activation(
                        h_sb[:, c, :], ps_h[:],
                        mybir.ActivationFunctionType.Copy,
                        scale=dinv_sb[:, c:c + 1])
                    for qq in range(NQ):
                        if c == QCHUNK[qq + 1] - 1:
                            ca, cb = QCHUNK[qq], QCHUNK[qq + 1]
                            nc.sync.dma_start(
                                out=hq[qq][:].rearrange(
                                    "(c p) f -> p c f", p=128),
                                in_=h_sb[:, ca:cb, :])
                            nc.gpsimd.collective_compute(
                                "AllGather", mybir.AluOpType.bypass,
                                replica_groups=[list(range(NC))],
                                ins=[hq[qq][:]],
                                outs=[tab[qq][:]],
                            )

                if dbg and l == 0:
                    nc.scalar.dma_start(
                        out=dbg_h[:], in_=h_sb[:].rearrange("p c f -> p (c f)"))
                    nc.scalar.dma_start(out=dbg_tab[:], in_=tab[0][:])

                # ---- edge rounds, one per src quarter ----
                for r in range(NQ):
                    live = {}
                    n_calls = (T * 128 + CALL - 1) // CALL
                    msg_tiles = []
                    for t in range(T):
                        if t % (CALL // 128) == 0:
                            k = t // (CALL // 128)
                            n_idx = min(CALL, T * 128 - k * CALL)
                            msg = msgp.tile([128, CALL // 128, 128], f16,
                                            tag="msg")
                            gcol0 = (r * T * 128 + k * CALL) // 16
                            nc.gpsimd.dma_gather(
                                out_ap=msg[:, :n_idx // 128, :],
                                in_ap=tab[r][:],
                                idxs_ap=gidx[:, gcol0:gcol0 + n_idx // 16],
                                num_idxs=n_idx,
                                num_idxs_reg=call_reg if n_idx == CALL
                                else tail_reg,
                                elem_size=128,
                                queue_num=(r * n_calls + k) % 4)
                            msg_tiles.append(msg)
                            if dbg and l == 0 and r == 0 and k == 0:
                                nc.scalar.dma_start(
                                    out=dbg_msg[:],
                                    in_=msg[:].rearrange("p c f -> p (c f)"))
                        # init / reload psum chunks first touched here
                        for k512 in (bc_t[t], bc_t[t] + 1):
                            if k512 in live or k512 >= NCH:
                                continue
                            if k512 == bc_t[t] + 1 and w0_t[t] == 0:
                                continue
                            if first_t[k512] != t:
                                continue
                            ps = pp_agg.tile([128, 512], f32, tag="agg")
                            live[k512] = ps
                            wk = chw[k512]
                            if r == 0:
                                for j in range(-(-wk // 128)):
                                    cc = k512 * 4 + j
                                    dg = diagp.tile([128, 128], f16, tag="dg")
                                    nc.vector.tensor_scalar(
                                        out=dg[:], in0=iota[:, 0:128],
                                        scalar1=pidx[:],
                                        scalar2=dinv_sb[:, cc:cc + 1],
                                        op0=mybir.AluOpType.is_equal,
                                        op1=mybir.AluOpType.mult)
                                    nc.tensor.matmul(
                                        ps[:, j * 128:(j + 1) * 128],
                                        lhsT=h_sb[:, cc, :], rhs=dg[:],
                                        start=j == 0, stop=False)
                            else:
                                nc.tensor.matmul(
                                    ps[:, 0:wk], lhsT=ident[:],
                                    rhs=pacc[:, k512 * 512:k512 * 512 + wk],
                                    start=True, stop=False)
                        # indicator + segment matmuls
                        ind = indp.tile([128, 512], f16, tag="ind")
                        gt = r * T + t
                        nc.vector.tensor_scalar(
                            out=ind[:], in0=iota[:, w0_t[t]:w0_t[t] + 512],
                            scalar1=dstrel[:, gt:gt + 1],
                            scalar2=sdw[:, gt:gt + 1],
                            op0=mybir.AluOpType.is_equal,
                            op1=mybir.AluOpType.mult)
                        if dbg and l == 0 and r == 0 and t == 0:
                            nc.scalar.dma_start(out=dbg_ind[:], in_=ind[:])
                        msg = msg_tiles[t // (CALL // 128)]
                        mj = t % (CALL // 128)
                        w0 = w0_t[t]
                        bc = bc_t[t]
                        wA = min(512, chw[bc]) - w0
                        nc.tensor.matmul(
                            live[bc][:, w0:w0 + wA],
                            lhsT=msg[:, mj, :], rhs=ind[:, 0:wA],
                            start=False, stop=last_t[bc] == t)
                        if w0 > 0 and bc + 1 < NCH:
                            wB = min(w0, chw[bc + 1])
                            nc.tensor.matmul(
                                live[bc + 1][:, 0:wB],
                                lhsT=msg[:, mj, :],
                                rhs=ind[:, 512 - w0:512 - w0 + wB],
                                start=False, stop=last_t[bc + 1] == t)
                        # evict finished chunks
                        for k512 in [k for k, lt in last_t.items()
                                     if lt == t and k in live]:
                            ps = live.pop(k512)
                            wk = chw[k512]
                            if r < NQ - 1:
                                nc.scalar.activation(
                                    pacc[:, k512 * 512:k512 * 512 + wk],
                                    ps[:, 0:wk],
                                    mybir.ActivationFunctionType.Copy)
                            else:
                                xf = evp.tile([128, 512], f16, tag="xf")
                                nc.scalar.activation(
                                    xf[:, 0:wk], ps[:, 0:wk],
                                    mybir.ActivationFunctionType.Relu,
                                    bias=b_sb[l][:] if has_bias else 0.0)
                                if not last:
                                    nc.vector.tensor_scalar_mul(
                                        xT[:, k512 * 512:k512 * 512 + wk],
                                        xf[:, 0:wk], gate_col[:])
                                else:
                                    xg = evp.tile([128, 512], f16, tag="xg")
                                    nc.vector.tensor_scalar_mul(
                                        xg[:, 0:wk], xf[:, 0:wk], gate_col[:])
                                    for j in range(-(-wk // 128)):
                                        cc = k512 * 4 + j
                                        ps_t = pp_t.tile(
                                            [128, 128], f32, tag="pst")
                                        nc.tensor.matmul(
                                            ps_t[:],
                                            lhsT=xg[:, j * 128:(j + 1) * 128],
                                            rhs=ident[:],
                                            start=True, stop=True)
                                        rowo = evp.tile(
                                            [128, 128], f32, tag="rowo")
                                        nc.scalar.activation(
                                            rowo[:], ps_t[:],
                                            mybir.ActivationFunctionType.Copy)
                                        nc.sync.dma_start(
                                            out=out_ext[
                                                cc * 128:(cc + 1) * 128, :],
                                            in_=rowo[:])
                    assert not live
                    if dbg and l == 0 and r == 0:
                        nc.scalar.dma_start(out=dbg_pacc[:], in_=pacc[:])
                if dbg and l == 0:
                    nc.scalar.dma_start(out=dbg_x1[:], in_=xT[:])

    mybir.codegen_inst_isa_subclasses(nc)


_CACHE = {}


def _get_program(plan, has_bias):
    key = (plan, has_bias, os.environ.get("KV2_DEBUG"))
    if key not in _CACHE:
        nc = bass.Bass(num_devices=NC, num_swdge_queues=4)
        _build(nc, plan, has_bias)
        _CACHE[key] = nc
    return _CACHE[key]


def _prepare(inputs):
    x = np.asarray(inputs["x"], np.float32)
    edge_index = np.asarray(inputs["edge_index"])
    ts = np.asarray(inputs["timestamp"], np.float32).reshape(-1)[0]
    Ws = [np.asarray(inputs[f"W{l}"], np.float32) for l in range(N_LAYERS)]
    bs = [np.asarray(inputs[f"b{l}"], np.float32) for l in range(N_LAYERS)]
    Wg1 = np.asarray(inputs["Wg1"], np.float32)
    bg1 = np.asarray(inputs["bg1"], np.float32)
    Wg2 = np.asarray(inputs["Wg2"], np.float32)
    bg2 = np.asarray(inputs["bg2"], np.float32)

    dinv, gidx, dstrel, sdw, plan = _prep_graph(edge_index)
    has_bias = any(np.abs(b).max() > 0 for b in bs)

    ident = np.eye(128, dtype=np.float16)
    iota = np.tile(np.arange(2048, dtype=np.float16), (128, 1))
    pidx = np.arange(128, dtype=np.float32).reshape(128, 1)
    in_maps = []
    for c in range(NC):
        lo = c * N_OWN
        hi = min((c + 1) * N_OWN, N_NODES)
        xb = np.zeros((N_OWN, 128), np.float16)
        xb[: hi - lo] = x[lo:hi].astype(np.float16)
        dv = np.ones(N_OWN, np.float32)
        dv[: hi - lo] = dinv[lo:hi]
        m = {
            "xT_in": np.ascontiguousarray(xb.T),
            "dinv_in": np.ascontiguousarray(dv.reshape(CHUNKS, 128).T),
            "pidx_in": pidx,
            "iota_in": iota,
            "gidx_in": _wrap_idx(gidx[c]),
            "dstrel_in": np.ascontiguousarray(
                dstrel[c].reshape(-1, 128).T),
            "sdw_in": np.ascontiguousarray(
                sdw[c].reshape(-1, 128).T),
            "ident_in": ident,
            "wg1_in": Wg1.reshape(128, 1),
            "bg1_in": bg1.reshape(128, 1),
            "wg2_in": np.ascontiguousarray(Wg2),
            "bg2_in": bg2.reshape(128, 1),
            "ts_in": np.full((128, 1), ts, np.float32),
        }
        for l in range(N_LAYERS):
            m[f"w{l}"] = Ws[l].astype(np.float16)
            if has_bias:
                m[f"b{l}"] = bs[l].astype(np.float32).reshape(128, 1)
        in_maps.append(m)
    return in_maps, plan, has_bias


def _run(inputs, trace=False):
    in_maps, plan, has_bias = _prepare(inputs)
    nc = _get_program(plan, has_bias)
    last_err = None
    for attempt in range(3):
        try:
            res = run_bass_kernel_spmd(
                nc, in_maps, core_ids=list(range(NC)), trace=trace)
            break
        except Exception as e:  # wedged device recovers on retry
            last_err = e
            if attempt == 2:
                raise
    blocks = [res.results[c]["out_ext"] for c in range(NC)]
    out = np.concatenate(blocks, axis=0)[:N_NODES].astype(np.float32)
    return out, res


def kernel(**inputs) -> np.ndarray:
    out, _ = _run(inputs, trace=False)
    return out


def kernel_traced(**inputs):
    return _run(inputs, trace=True)


# revision 3
# speedup vs baseline: 1.0338x; 1.0338x over previous
"""DynamicGCN (3-layer GCN + temporal gate) on 8 trn2 NeuronCores via Bass.

v2: feature-major edge aggregation directly in PSUM.

Nodes are partitioned contiguously across the 8 cores (12544 rows each,
padded). Per layer:

  1. h' = dinv * (x @ W) for own nodes (PE matmul node-major, ACT eviction
     applies dinv), kept in SBUF (h_sb) and staged per src-quarter to DRAM.
  2. Four chunked AllGathers (one per src-quarter) build int16-indexable
     gather tables (<= 25600 rows each).
  3. Per quarter ("round"), edges sorted by dst are packed into fixed
     dst-windows: tile t covers dst in [W*t, W*t+512). dma_gather pulls the
     128 src rows of each tile; a DVE tensor_scalar generates the indicator
     rhs on the fly: ind[e, j] = (iota[j] == dstrel[e]) * dinv[dst[e]].
     One or two PE matmuls per tile accumulate msg^T @ ind into the
     feature-major PSUM tile of the 512-node chunk(s) the window covers.
  4. The self-loop term is injected by diagonal init matmuls (start=True);
     between rounds partial sums bounce through SBUF (pacc) via identity
     reload matmuls. After round 3: relu (+bias) * gate evicts straight
     into xT (already transposed for the next layer's matmul). The last
     layer transposes back to node-major and DMAs to the output.

The temporal gate MLP runs once on-device at kernel start.
"""
import sys, os, types

for _p in ("/opt/trn_rl_repo", os.path.dirname(os.path.abspath(__file__))):
    if _p not in sys.path:
        sys.path.insert(0, _p)

import numpy as np

# ---------------------------------------------------------------- shims ----
def _install_shims():
    # NTFF profile hook (missing module in this container; used for trace=True)
    if "antenv.axon_hooks" not in sys.modules:
        try:
            import antenv
            from trn_agent_boot.trn_boot import _ntff_profile_via_ctypes

            mod = types.ModuleType("antenv.axon_hooks")
            _state = {"hook": None}
            mod.set_axon_ntff_profile_hook = lambda h: _state.__setitem__("hook", h)
            mod.get_axon_ntff_profile_hook = lambda: _state["hook"]
            sys.modules["antenv.axon_hooks"] = mod
            antenv.axon_hooks = mod
            if os.path.exists("/opt/axon/libaxon_pjrt.so"):
                mod.set_axon_ntff_profile_hook(
                    _ntff_profile_via_ctypes("/opt/axon/libaxon_pjrt.so")
                )
        except Exception:
            pass

    # walrus in this container rejects >1 sync wait per instruction; split
    # extra waits onto same-engine NoOps (identical semantics).
    import concourse.bass as bass
    import orjson

    if getattr(bass.Bass.to_json_bytes, "_waitsplit", False):
        return

    orig = bass.Bass.to_json_bytes

    def _split(j):
        ctr = 0
        for fn in j.get("functions", []):
            for bb in fn.get("blocks", []):
                out, changed = [], False
                for ins in bb.get("instructions", []):
                    si = ins.get("sync_info")
                    waits = (si or {}).get("on_wait") or []
                    if len(waits) > 1 and ins.get("engine") not in (None, "Unassigned"):
                        for w in waits[:-1]:
                            ctr += 1
                            out.append({
                                "debug": ins.get("debug", 0), "engine": ins["engine"],
                                "ins": [], "outs": [], "name": f"I-wsplit-{ctr}",
                                "opcode": "NoOp",
                                "sync_info": {"on_update": [], "on_wait": [w]},
                            })
                        si["on_wait"] = [waits[-1]]
                        changed = True
                    out.append(ins)
                if changed:
                    bb["instructions"] = out
        return j

    def to_json_bytes(self):
        return orjson.dumps(_split(orjson.loads(orig(self))))

    to_json_bytes._waitsplit = True
    bass.Bass.to_json_bytes = to_json_bytes


_install_shims()

import concourse.bass as bass
import concourse.mybir as mybir
import concourse.tile as tile
from concourse import library_config
from concourse.bass_utils import run_bass_kernel_spmd

f16 = mybir.dt.float16
f32 = mybir.dt.float32
i16 = mybir.dt.int16

# ---------------------------------------------------------- problem dims ---
N_NODES = 100000
N_EDGES = 600000
D = 128
N_LAYERS = 3
NC = 8
N_OWN = 12544                 # padded rows per core (= 98*128)
NQ = 4                        # src quarters (AG chunks / gather tables)
QSTART = [0, 3200, 6400, 9600, 12544]
QSIZE = [3200, 3200, 3200, 2944]
CHUNKS = 98                   # 128-row matmul chunks per core
QCHUNK = [0, 25, 50, 75, 98]  # 128-chunk boundaries of quarters
NCH = 25                      # 512-node psum chunks (24 full + 1 of 256)
CALL = 256                    # rows per dma_gather call
MAXSPAN = 640                 # max per-tile dst span (envelope over cores)


def _wrap_idx(vals):
    """int16 stream -> [128, n/16] tile layout (16-partition wrap, replicated
    for the 8 Q7 cores). vals length must be a multiple of 16."""
    a = np.asarray(vals, np.int16).reshape(-1, 16).T  # [16, n/16]
    return np.tile(a, (8, 1)).copy()


def _prep_graph(edge_index):
    """Partition/sort edges; strict 128-edge tiles with per-(q,t) static
    column spans (envelope over the 8 cores).

    Returns (dinv, gidx, dstrel, sdw, plan) where plan = per-quarter tuple of
    (a, b) node spans per tile; gidx/dstrel/sdw are [NC, sum_q T_q * 128].
    """
    src = np.asarray(edge_index[0], np.int64)
    dst = np.asarray(edge_index[1], np.int64)
    deg = np.bincount(dst, minlength=N_NODES).astype(np.float32) + 1.0
    dinv = (1.0 / np.sqrt(deg)).astype(np.float32)

    core = dst // N_OWN
    rank = src // N_OWN
    i_loc = src % N_OWN
    q = np.searchsorted(np.asarray(QSTART[1:4]), i_loc, side="right")
    qs = np.asarray([QSIZE[j] for j in range(NQ)])
    qst = np.asarray([QSTART[j] for j in range(NQ)])
    tabrow = rank * qs[q] + (i_loc - qst[q])
    dst_loc = dst % N_OWN
    sv = dinv[dst]

    # sorted per (core, quarter) streams
    streams = {}
    for c in range(NC):
        mc = core == c
        for qq in range(NQ):
            m = mc & (q == qq)
            order = np.argsort(dst_loc[m], kind="stable")
            streams[(c, qq)] = (tabrow[m][order], dst_loc[m][order],
                                sv[m][order])
    T_q = [max(-(-streams[(c, qq)][1].size // 128) for c in range(NC))
           for qq in range(NQ)]
    t_tot = sum(T_q)
    gidx = np.zeros((NC, t_tot * 128), np.int16)
    dstrel = np.full((NC, t_tot * 128), -1.0, np.float32)
    sdw = np.zeros((NC, t_tot * 128), np.float32)
    plan = []
    off = 0
    for qq in range(NQ):
        spans = []
        for t in range(T_q[qq]):
            a, b = N_OWN, 0
            for c in range(NC):
                dl = streams[(c, qq)][1]
                if t * 128 < dl.size:
                    a = min(a, int(dl[t * 128]))
                    b = max(b, int(dl[min(t * 128 + 127, dl.size - 1)]) + 1)
            if b <= a:
                spans.append((0, 0))
                continue
            assert b - a <= MAXSPAN, f"tile span {b - a} exceeds {MAXSPAN}"
            base = 512 * (a // 512)
            spans.append((a, b))
            for c in range(NC):
                tr, dl, svq = streams[(c, qq)]
                e0 = t * 128
                n = min(128, dl.size - e0)
                if n <= 0:
                    continue
                s = off + t * 128
                gidx[c, s:s + n] = tr[e0:e0 + n]
                dstrel[c, s:s + n] = dl[e0:e0 + n] - base
                sdw[c, s:s + n] = svq[e0:e0 + n]
        plan.append(tuple(spans))
        off += T_q[qq] * 128
    return dinv, gidx, dstrel, sdw, tuple(plan)


def _build(nc_prog, plan, has_bias):
    nc = nc_prog
    T_q = [len(spans) for spans in plan]
    t_tot = sum(T_q)
    # ---------------- I/O ----------------
    xT_in = nc.dram_tensor("xT_in", [128, N_OWN], f16, kind="ExternalInput")
    w_in = [nc.dram_tensor(f"w{l}", [128, 128], f16, kind="ExternalInput")
            for l in range(N_LAYERS)]
    dinv_in = nc.dram_tensor("dinv_in", [128, CHUNKS], f32, kind="ExternalInput")
    pidx_in = nc.dram_tensor("pidx_in", [128, 1], f32, kind="ExternalInput")
    iota_in = nc.dram_tensor("iota_in", [128, 2048], f16, kind="ExternalInput")
    gidx_in = nc.dram_tensor("gidx_in", [128, t_tot * 8], i16, kind="ExternalInput")
    dstrel_in = nc.dram_tensor("dstrel_in", [128, t_tot], f32, kind="ExternalInput")
    sdw_in = nc.dram_tensor("sdw_in", [128, t_tot], f32, kind="ExternalInput")
    ident_in = nc.dram_tensor("ident_in", [128, 128], f16, kind="ExternalInput")
    # gate MLP params
    wg1_in = nc.dram_tensor("wg1_in", [128, 1], f32, kind="ExternalInput")
    bg1_in = nc.dram_tensor("bg1_in", [128, 1], f32, kind="ExternalInput")
    wg2_in = nc.dram_tensor("wg2_in", [128, 128], f32, kind="ExternalInput")
    bg2_in = nc.dram_tensor("bg2_in", [128, 1], f32, kind="ExternalInput")
    ts_in = nc.dram_tensor("ts_in", [128, 1], f32, kind="ExternalInput")
    b_in = None
    if has_bias:
        b_in = [nc.dram_tensor(f"b{l}", [128, 1], f32, kind="ExternalInput")
                for l in range(N_LAYERS)]
    out_ext = nc.dram_tensor("out_ext", [N_OWN, 128], f32, kind="ExternalOutput")
    dbg = os.environ.get("KV2_DEBUG") == "1"
    if dbg:
        dbg_h = nc.dram_tensor("dbg_h", [128, CHUNKS * 128], f16,
                               kind="ExternalOutput")
        dbg_tab = nc.dram_tensor("dbg_tab", [NC * QSIZE[0], 128], f16,
                                 kind="ExternalOutput")
        dbg_x1 = nc.dram_tensor("dbg_x1", [128, N_OWN], f16,
                                kind="ExternalOutput")
        dbg_pacc = nc.dram_tensor("dbg_pacc", [128, N_OWN], f16,
                                  kind="ExternalOutput")

    # ---------------- internal DRAM ----------------
    hq = [nc.dram_tensor(f"hq{qq}", [QSIZE[qq], 128], f16) for qq in range(NQ)]
    tab = [nc.dram_tensor(f"tab{qq}", [NC * QSIZE[qq], 128], f16,
                          addr_space="Shared") for qq in range(NQ)]

    # per-quarter chunk touch maps (static, identical on all cores)
    chw = [512] * (NCH - 1) + [N_OWN - 512 * (NCH - 1)]  # last chunk 256
    first_t = []   # per q: {chunk: first tile}
    last_t = []    # per q: {chunk: last tile}
    for qq in range(NQ):
        ft, lt = {}, {}
        for t, (a, b) in enumerate(plan[qq]):
            if b <= a:
                continue
            for k in range(a // 512, (b - 1) // 512 + 1):
                if k not in ft:
                    ft[k] = t
                lt[k] = t
        first_t.append(ft)
        last_t.append(lt)

    with tile.TileContext(nc) as tc:
        with (
            tc.tile_pool(name="const", bufs=1) as cp,
            tc.tile_pool(name="msgp", bufs=32) as msgp,
            tc.tile_pool(name="indp", bufs=32) as indp,
            tc.tile_pool(name="diagp", bufs=12) as diagp,
            tc.tile_pool(name="evp", bufs=8) as evp,
            tc.tile_pool(name="psum_h", bufs=2, space="PSUM") as pp_h,
            tc.tile_pool(name="psum_agg", bufs=4, space="PSUM") as pp_agg,
            tc.tile_pool(name="psum_t", bufs=2, space="PSUM") as pp_t,
        ):
            nc.gpsimd.load_library(library_config.mlp)
            nregs = {CALL: nc.gpsimd.to_reg(CALL)}
            for qq in range(NQ):
                tn = T_q[qq] * 128 - (T_q[qq] * 128 // CALL) * CALL
                if tn and tn not in nregs:
                    nregs[tn] = nc.gpsimd.to_reg(tn)

            # persistent SBUF
            xT = cp.tile([128, N_OWN], f16)
            nc.sync.dma_start(out=xT[:], in_=xT_in[:])
            wt = []
            for l in range(N_LAYERS):
                w = cp.tile([128, 128], f16, tag=f"w{l}")
                nc.sync.dma_start(out=w[:], in_=w_in[l][:])
                wt.append(w)
            dinv_sb = cp.tile([128, CHUNKS], f32)
            nc.sync.dma_start(out=dinv_sb[:], in_=dinv_in[:])
            pidx = cp.tile([128, 1], f32)
            nc.sync.dma_start(out=pidx[:], in_=pidx_in[:])
            iota = cp.tile([128, 2048], f16)
            nc.sync.dma_start(out=iota[:], in_=iota_in[:])
            gidx = cp.tile([128, t_tot * 8], i16)
            nc.sync.dma_start(out=gidx[:], in_=gidx_in[:])
            dstrel = cp.tile([128, t_tot], f32)
            nc.sync.dma_start(out=dstrel[:], in_=dstrel_in[:])
            sdw = cp.tile([128, t_tot], f32)
            nc.sync.dma_start(out=sdw[:], in_=sdw_in[:])
            ident = cp.tile([128, 128], f16)
            nc.sync.dma_start(out=ident[:], in_=ident_in[:])
            h_sb = cp.tile([128, CHUNKS, 128], f16)
            pacc = cp.tile([128, N_OWN], f16)
            b_sb = []
            if has_bias:
                for l in range(N_LAYERS):
                    bt = cp.tile([128, 1], f32, tag=f"b{l}")
                    nc.sync.dma_start(out=bt[:], in_=b_in[l][:])
                    b_sb.append(bt)

            # ---------------- temporal gate ----------------
            wg1 = cp.tile([128, 1], f32)
            nc.sync.dma_start(out=wg1[:], in_=wg1_in[:])
            bg1 = cp.tile([128, 1], f32)
            nc.sync.dma_start(out=bg1[:], in_=bg1_in[:])
            wg2 = cp.tile([128, 128], f32)
            nc.sync.dma_start(out=wg2[:], in_=wg2_in[:])
            bg2 = cp.tile([128, 1], f32)
            nc.sync.dma_start(out=bg2[:], in_=bg2_in[:])
            tsr = cp.tile([128, 1], f32)
            nc.sync.dma_start(out=tsr[:], in_=ts_in[:])

            tmp1 = cp.tile([128, 1], f32, tag="g1")
            nc.vector.tensor_mul(tmp1[:], wg1[:], tsr[:])
            tanh1 = cp.tile([128, 1], f32, tag="g2")
            nc.scalar.activation(
                tanh1[:], tmp1[:], mybir.ActivationFunctionType.Tanh, bias=bg1[:])
            ps_g = pp_h.tile([128, 1], f32, tag="psh")
            nc.tensor.matmul(ps_g[:], lhsT=wg2[:], rhs=tanh1[:], start=True, stop=True)
            gate_col = cp.tile([128, 1], f32, tag="gcol")
            nc.scalar.activation(
                gate_col[:], ps_g[:], mybir.ActivationFunctionType.Sigmoid,
                bias=bg2[:])

            # ---------------- layers ----------------
            for l in range(N_LAYERS):
                last = l == N_LAYERS - 1
                # h' = dinv * (x @ W) into h_sb; DMA per quarter + AllGather
                for c in range(CHUNKS):
                    ps_h = pp_h.tile([128, 128], f32, tag="psh")
                    nc.tensor.matmul(
                        ps_h[:], lhsT=xT[:, c * 128:(c + 1) * 128],
                        rhs=wt[l][:], start=True, stop=True)
                    nc.scalar.activation(
                        h_sb[:, c, :], ps_h[:],
                        mybir.ActivationFunctionType.Copy,
                        scale=dinv_sb[:, c:c + 1])
                    for qq in range(NQ):
                        if c == QCHUNK[qq + 1] - 1:
                            ca, cb = QCHUNK[qq], QCHUNK[qq + 1]
                            nc.sync.dma_start(
                                out=hq[qq][:].rearrange(
                                    "(c p) f -> p c f", p=128),
                                in_=h_sb[:, ca:cb, :])
                            nc.gpsimd.collective_compute(
                                "AllGather", mybir.AluOpType.bypass,
                                replica_groups=[list(range(NC))],
                                ins=[hq[qq][:]],
                                outs=[tab[qq][:]],
                            )

                if dbg and l == 0:
                    nc.scalar.dma_start(
                        out=dbg_h[:], in_=h_sb[:].rearrange("p c f -> p (c f)"))
                    nc.scalar.dma_start(out=dbg_tab[:], in_=tab[0][:])

                # ---- edge rounds, one per src quarter ----
                for r in range(NQ):
                    live = {}
                    n_calls = (T * 128 + CALL - 1) // CALL
                    msg_tiles = []
                    for t in range(T):
                        if t % (CALL // 128) == 0:
                            k = t // (CALL // 128)
                            n_idx = min(CALL, T * 128 - k * CALL)
                            msg = msgp.tile([128, CALL // 128, 128], f16,
                                            tag="msg")
                            gcol0 = (r * T * 128 + k * CALL) // 16
                            nc.gpsimd.dma_gather(
                                out_ap=msg[:, :n_idx // 128, :],
                                in_ap=tab[r][:],
                                idxs_ap=gidx[:, gcol0:gcol0 + n_idx // 16],
                                num_idxs=n_idx,
                                num_idxs_reg=call_reg if n_idx == CALL
                                else tail_reg,
                                elem_size=128,
                                queue_num=(r * n_calls + k) % 4)
                            msg_tiles.append(msg)
                            if dbg and l == 0 and r == 0 and k == 0:
                                nc.scalar.dma_start(
                                    out=dbg_msg[:],
                                    in_=msg[:].rearrange("p c f -> p (c f)"))
                        # init / reload psum chunks first touched here
                        for k512 in (bc_t[t], bc_t[t] + 1):
                            if k512 in live or k512 >= NCH:
                                continue
                            if k512 == bc_t[t] + 1 and w0_t[t] == 0:
                                continue
                            if first_t[k512] != t:
                                continue
                            ps = pp_agg.tile([128, 512], f32, tag="agg")
                            live[k512] = ps
                            wk = chw[k512]
                            if r == 0:
                                for j in range(-(-wk // 128)):
                                    cc = k512 * 4 + j
                                    dg = diagp.tile([128, 128], f16, tag="dg")
                                    nc.vector.tensor_scalar(
                                        out=dg[:], in0=iota[:, 0:128],
                                        scalar1=pidx[:],
                                        scalar2=dinv_sb[:, cc:cc + 1],
                                        op0=mybir.AluOpType.is_equal,
                                        op1=mybir.AluOpType.mult)
                                    nc.tensor.matmul(
                                        ps[:, j * 128:(j + 1) * 128],
                                        lhsT=h_sb[:, cc, :], rhs=dg[:],
                                        start=j == 0, stop=False)
                            else:
                                nc.tensor.matmul(
                                    ps[:, 0:wk], lhsT=ident[:],
                                    rhs=pacc[:, k512 * 512:k512 * 512 + wk],
                                    start=True, stop=False)
                        # indicator + segment matmuls
                        ind = indp.tile([128, 512], f16, tag="ind")
                        gt = r * T + t
                        nc.vector.tensor_scalar(
                            out=ind[:], in0=iota[:, w0_t[t]:w0_t[t] + 512],
                            scalar1=dstrel[:, gt:gt + 1],
                            scalar2=sdw[:, gt:gt + 1],
                            op0=mybir.AluOpType.is_equal,
                            op1=mybir.AluOpType.mult)
                        if dbg and l == 0 and r == 0 and t == 0:
                            nc.scalar.dma_start(out=dbg_ind[:], in_=ind[:])
                        msg = msg_tiles[t // (CALL // 128)]
                        mj = t % (CALL // 128)
                        w0 = w0_t[t]
                        bc = bc_t[t]
                        wA = min(512, chw[bc]) - w0
                        nc.tensor.matmul(
                            live[bc][:, w0:w0 + wA],
                            lhsT=msg[:, mj, :], rhs=ind[:, 0:wA],
                            start=False, stop=last_t[bc] == t)
                        if w0 > 0 and bc + 1 < NCH:
                            wB = min(w0, chw[bc + 1])
                            nc.tensor.matmul(
                                live[bc + 1][:, 0:wB],
                                lhsT=msg[:, mj, :],
                                rhs=ind[:, 512 - w0:512 - w0 + wB],
                                start=False, stop=last_t[bc + 1] == t)
                        # evict finished chunks
                        for k512 in [k for k, lt in last_t.items()
                                     if lt == t and k in live]:
                            ps = live.pop(k512)
                            wk = chw[k512]
                            if r < NQ - 1:
                                nc.scalar.activation(
                                    pacc[:, k512 * 512:k512 * 512 + wk],
                                    ps[:, 0:wk],
                                    mybir.ActivationFunctionType.Copy)
                            else:
                                xf = evp.tile([128, 512], f16, tag="xf")
                                nc.scalar.activation(
                                    xf[:, 0:wk], ps[:, 0:wk],
                                    mybir.ActivationFunctionType.Relu,
                                    bias=b_sb[l][:] if has_bias else 0.0)
                                if not last:
                                    nc.vector.tensor_scalar_mul(
                                        xT[:, k512 * 512:k512 * 512 + wk],
                                        xf[:, 0:wk], gate_col[:])
                                else:
                                    xg = evp.tile([128, 512], f16, tag="xg")
                                    nc.vector.tensor_scalar_mul(
                                        xg[:, 0:wk], xf[:, 0:wk], gate_col[:])
                                    for j in range(-(-wk // 128)):
                                        cc = k512 * 4 + j
                                        ps_t = pp_t.tile(
                                            [128, 128], f32, tag="pst")
                                        nc.tensor.matmul(
                                            ps_t[:],
                                            lhsT=xg[:, j * 128:(j + 1) * 128],
                                            rhs=ident[:],
                                            start=True, stop=True)
                                        rowo = evp.tile(
                                            [128, 128], f32, tag="rowo")
                                        nc.scalar.activation(
                                            rowo[:], ps_t[:],
                                            mybir.ActivationFunctionType.Copy)
                                        nc.sync.dma_start(
                                            out=out_ext[
                                                cc * 128:(cc + 1) * 128, :],
                                            in_=rowo[:])
                    assert not live
                    if dbg and l == 0 and r == 0:
                        nc.scalar.dma_start(out=dbg_pacc[:], in_=pacc[:])
                if dbg and l == 0:
                    nc.scalar.dma_start(out=dbg_x1[:], in_=xT[:])

    mybir.codegen_inst_isa_subclasses(nc)


_CACHE = {}


def _get_program(plan, has_bias):
    key = (plan, has_bias, os.environ.get("KV2_DEBUG"))
    if key not in _CACHE:
        nc = bass.Bass(num_devices=NC, num_swdge_queues=4)
        _build(nc, plan, has_bias)
        _CACHE[key] = nc
    return _CACHE[key]


def _prepare(inputs):
    x = np.asarray(inputs["x"], np.float32)
    edge_index = np.asarray(inputs["edge_index"])
    ts = np.asarray(inputs["timestamp"], np.float32).reshape(-1)[0]
    Ws = [np.asarray(inputs[f"W{l}"], np.float32) for l in range(N_LAYERS)]
    bs = [np.asarray(inputs[f"b{l}"], np.float32) for l in range(N_LAYERS)]
    Wg1 = np.asarray(inputs["Wg1"], np.float32)
    bg1 = np.asarray(inputs["bg1"], np.float32)
    Wg2 = np.asarray(inputs["Wg2"], np.float32)
    bg2 = np.asarray(inputs["bg2"], np.float32)

    dinv, gidx, dstrel, sdw, plan = _prep_graph(edge_index)
    has_bias = any(np.abs(b).max() > 0 for b in bs)

    ident = np.eye(128, dtype=np.float16)
    iota = np.tile(np.arange(2048, dtype=np.float16), (128, 1))
    pidx = np.arange(128, dtype=np.float32).reshape(128, 1)
    in_maps = []
    for c in range(NC):
        lo = c * N_OWN
        hi = min((c + 1) * N_OWN, N_NODES)
        xb = np.zeros((N_OWN, 128), np.float16)
        xb[: hi - lo] = x[lo:hi].astype(np.float16)
        dv = np.ones(N_OWN, np.float32)
        dv[: hi - lo] = dinv[lo:hi]
        m = {
            "xT_in": np.ascontiguousarray(xb.T),
            "dinv_in": np.ascontiguousarray(dv.reshape(CHUNKS, 128).T),
            "pidx_in": pidx,
            "iota_in": iota,
            "gidx_in": _wrap_idx(gidx[c]),
            "dstrel_in": np.ascontiguousarray(
                dstrel[c].reshape(-1, 128).T),
            "sdw_in": np.ascontiguousarray(
                sdw[c].reshape(-1, 128).T),
            "ident_in": ident,
            "wg1_in": Wg1.reshape(128, 1),
            "bg1_in": bg1.reshape(128, 1),
            "wg2_in": np.ascontiguousarray(Wg2),
            "bg2_in": bg2.reshape(128, 1),
            "ts_in": np.full((128, 1), ts, np.float32),
        }
        for l in range(N_LAYERS):
            m[f"w{l}"] = Ws[l].astype(np.float16)
            if has_bias:
                m[f"b{l}"] = bs[l].astype(np.float32).reshape(128, 1)
        in_maps.append(m)
    return in_maps, plan, has_bias


def _run(inputs, trace=False):
    in_maps, plan, has_bias = _prepare(inputs)
    nc = _get_program(plan, has_bias)
    last_err = None
    for attempt in range(3):
        try:
            res = run_bass_kernel_spmd(
                nc, in_maps, core_ids=list(range(NC)), trace=trace)
            break
        except Exception as e:  # wedged device recovers on retry
            last_err = e
            if attempt == 2:
                raise
    blocks = [res.results[c]["out_ext"] for c in range(NC)]
    out = np.concatenate(blocks, axis=0)[:N_NODES].astype(np.float32)
    return out, res


def kernel(**inputs) -> np.ndarray:
    out, _ = _run(inputs, trace=False)
    return out


def kernel_traced(**inputs):
    return _run(inputs, trace=True)


# revision 4
# speedup vs baseline: 1.1308x; 1.0938x over previous
"""DynamicGCN (3-layer GCN + temporal gate) on 8 trn2 NeuronCores via Bass.

v2: feature-major edge aggregation directly in PSUM.

Nodes are partitioned contiguously across the 8 cores (12544 rows each,
padded). Per layer:

  1. h' = dinv * (x @ W) for own nodes (PE matmul node-major, ACT eviction
     applies dinv), kept in SBUF (h_sb) and staged per src-quarter to DRAM.
  2. Four chunked AllGathers (one per src-quarter) build int16-indexable
     gather tables (<= 25600 rows each).
  3. Per quarter ("round"), edges sorted by dst are packed into fixed
     dst-windows: tile t covers dst in [W*t, W*t+512). dma_gather pulls the
     128 src rows of each tile; a DVE tensor_scalar generates the indicator
     rhs on the fly: ind[e, j] = (iota[j] == dstrel[e]) * dinv[dst[e]].
     One or two PE matmuls per tile accumulate msg^T @ ind into the
     feature-major PSUM tile of the 512-node chunk(s) the window covers.
  4. The self-loop term is injected by diagonal init matmuls (start=True);
     between rounds partial sums bounce through SBUF (pacc) via identity
     reload matmuls. After round 3: relu (+bias) * gate evicts straight
     into xT (already transposed for the next layer's matmul). The last
     layer transposes back to node-major and DMAs to the output.

The temporal gate MLP runs once on-device at kernel start.
"""
import sys, os, types

for _p in ("/opt/trn_rl_repo", os.path.dirname(os.path.abspath(__file__))):
    if _p not in sys.path:
        sys.path.insert(0, _p)

import numpy as np

# ---------------------------------------------------------------- shims ----
def _install_shims():
    # NTFF profile hook (missing module in this container; used for trace=True)
    if "antenv.axon_hooks" not in sys.modules:
        try:
            import antenv
            from trn_agent_boot.trn_boot import _ntff_profile_via_ctypes

            mod = types.ModuleType("antenv.axon_hooks")
            _state = {"hook": None}
            mod.set_axon_ntff_profile_hook = lambda h: _state.__setitem__("hook", h)
            mod.get_axon_ntff_profile_hook = lambda: _state["hook"]
            sys.modules["antenv.axon_hooks"] = mod
            antenv.axon_hooks = mod
            if os.path.exists("/opt/axon/libaxon_pjrt.so"):
                mod.set_axon_ntff_profile_hook(
                    _ntff_profile_via_ctypes("/opt/axon/libaxon_pjrt.so")
                )
        except Exception:
            pass

    # walrus in this container rejects >1 sync wait per instruction; split
    # extra waits onto same-engine NoOps (identical semantics).
    import concourse.bass as bass
    import orjson

    if getattr(bass.Bass.to_json_bytes, "_waitsplit", False):
        return

    orig = bass.Bass.to_json_bytes

    def _split(j):
        ctr = 0
        for fn in j.get("functions", []):
            for bb in fn.get("blocks", []):
                out, changed = [], False
                for ins in bb.get("instructions", []):
                    si = ins.get("sync_info")
                    waits = (si or {}).get("on_wait") or []
                    if len(waits) > 1 and ins.get("engine") not in (None, "Unassigned"):
                        for w in waits[:-1]:
                            ctr += 1
                            out.append({
                                "debug": ins.get("debug", 0), "engine": ins["engine"],
                                "ins": [], "outs": [], "name": f"I-wsplit-{ctr}",
                                "opcode": "NoOp",
                                "sync_info": {"on_update": [], "on_wait": [w]},
                            })
                        si["on_wait"] = [waits[-1]]
                        changed = True
                    out.append(ins)
                if changed:
                    bb["instructions"] = out
        return j

    def to_json_bytes(self):
        return orjson.dumps(_split(orjson.loads(orig(self))))

    to_json_bytes._waitsplit = True
    bass.Bass.to_json_bytes = to_json_bytes


_install_shims()

import concourse.bass as bass
import concourse.mybir as mybir
import concourse.tile as tile
from concourse import library_config
from concourse.bass_utils import run_bass_kernel_spmd

f16 = mybir.dt.float16
f32 = mybir.dt.float32
i16 = mybir.dt.int16

# ---------------------------------------------------------- problem dims ---
N_NODES = 100000
N_EDGES = 600000
D = 128
N_LAYERS = 3
NC = 8
N_OWN = 12544                 # padded rows per core (= 98*128)
NQ = 4                        # src quarters (AG chunks / gather tables)
QSTART = [0, 3200, 6400, 9600, 12544]
QSIZE = [3200, 3200, 3200, 2944]
CHUNKS = 98                   # 128-row matmul chunks per core
QCHUNK = [0, 25, 50, 75, 98]  # 128-chunk boundaries of quarters
NCH = 25                      # 512-node psum chunks (24 full + 1 of 256)
CALL = 384                    # rows per dma_gather call
MAXSPAN = 640                 # max per-tile dst span (envelope over cores)


def _wrap_idx(vals):
    """int16 stream -> [128, n/16] tile layout (16-partition wrap, replicated
    for the 8 Q7 cores). vals length must be a multiple of 16."""
    a = np.asarray(vals, np.int16).reshape(-1, 16).T  # [16, n/16]
    return np.tile(a, (8, 1)).copy()


def _prep_graph(edge_index):
    """Partition/sort edges; strict 128-edge tiles with per-(q,t) static
    column spans (envelope over the 8 cores).

    Returns (dinv, gidx, dstrel, sdw, plan) where plan = per-quarter tuple of
    (a, b) node spans per tile; gidx/dstrel/sdw are [NC, sum_q T_q * 128].
    """
    src = np.asarray(edge_index[0], np.int64)
    dst = np.asarray(edge_index[1], np.int64)
    deg = np.bincount(dst, minlength=N_NODES).astype(np.float32) + 1.0
    dinv = (1.0 / np.sqrt(deg)).astype(np.float32)

    core = dst // N_OWN
    rank = src // N_OWN
    i_loc = src % N_OWN
    q = np.searchsorted(np.asarray(QSTART[1:4]), i_loc, side="right")
    qs = np.asarray([QSIZE[j] for j in range(NQ)])
    qst = np.asarray([QSTART[j] for j in range(NQ)])
    tabrow = rank * qs[q] + (i_loc - qst[q])
    dst_loc = dst % N_OWN
    sv = dinv[dst]

    # sorted per (core, quarter) streams
    streams = {}
    for c in range(NC):
        mc = core == c
        for qq in range(NQ):
            m = mc & (q == qq)
            order = np.argsort(dst_loc[m], kind="stable")
            streams[(c, qq)] = (tabrow[m][order], dst_loc[m][order],
                                sv[m][order])
    T_q = [max(-(-streams[(c, qq)][1].size // 128) for c in range(NC))
           for qq in range(NQ)]
    t_tot = sum(T_q)
    gidx = np.zeros((NC, t_tot * 128), np.int16)
    dstrel = np.full((NC, t_tot * 128), -1.0, np.float32)
    sdw = np.zeros((NC, t_tot * 128), np.float32)
    plan = []
    off = 0
    for qq in range(NQ):
        spans = []
        for t in range(T_q[qq]):
            a, b = N_OWN, 0
            for c in range(NC):
                dl = streams[(c, qq)][1]
                if t * 128 < dl.size:
                    a = min(a, int(dl[t * 128]))
                    b = max(b, int(dl[min(t * 128 + 127, dl.size - 1)]) + 1)
            if b <= a:
                spans.append((0, 0))
                continue
            assert b - a <= MAXSPAN, f"tile span {b - a} exceeds {MAXSPAN}"
            base = 512 * (a // 512)
            spans.append((a, b))
            for c in range(NC):
                tr, dl, svq = streams[(c, qq)]
                e0 = t * 128
                n = min(128, dl.size - e0)
                if n <= 0:
                    continue
                s = off + t * 128
                gidx[c, s:s + n] = tr[e0:e0 + n]
                dstrel[c, s:s + n] = dl[e0:e0 + n] - base
                sdw[c, s:s + n] = svq[e0:e0 + n]
        plan.append(tuple(spans))
        off += T_q[qq] * 128
    return dinv, gidx, dstrel, sdw, tuple(plan)


def _build(nc_prog, plan, has_bias):
    nc = nc_prog
    T_q = [len(spans) for spans in plan]
    t_tot = sum(T_q)
    # ---------------- I/O ----------------
    xT_in = nc.dram_tensor("xT_in", [128, N_OWN], f16, kind="ExternalInput")
    w_in = [nc.dram_tensor(f"w{l}", [128, 128], f16, kind="ExternalInput")
            for l in range(N_LAYERS)]
    dinv_in = nc.dram_tensor("dinv_in", [128, CHUNKS], f32, kind="ExternalInput")
    pidx_in = nc.dram_tensor("pidx_in", [128, 1], f32, kind="ExternalInput")
    iota_in = nc.dram_tensor("iota_in", [128, 2048], f16, kind="ExternalInput")
    gidx_in = nc.dram_tensor("gidx_in", [128, t_tot * 8], i16, kind="ExternalInput")
    dstrel_in = nc.dram_tensor("dstrel_in", [128, t_tot], f32, kind="ExternalInput")
    sdw_in = nc.dram_tensor("sdw_in", [128, t_tot], f32, kind="ExternalInput")
    ident_in = nc.dram_tensor("ident_in", [128, 128], f16, kind="ExternalInput")
    # gate MLP params
    wg1_in = nc.dram_tensor("wg1_in", [128, 1], f32, kind="ExternalInput")
    bg1_in = nc.dram_tensor("bg1_in", [128, 1], f32, kind="ExternalInput")
    wg2_in = nc.dram_tensor("wg2_in", [128, 128], f32, kind="ExternalInput")
    bg2_in = nc.dram_tensor("bg2_in", [128, 1], f32, kind="ExternalInput")
    ts_in = nc.dram_tensor("ts_in", [128, 1], f32, kind="ExternalInput")
    b_in = None
    if has_bias:
        b_in = [nc.dram_tensor(f"b{l}", [128, 1], f32, kind="ExternalInput")
                for l in range(N_LAYERS)]
    out_ext = nc.dram_tensor("out_ext", [N_OWN, 128], f32, kind="ExternalOutput")
    dbg = os.environ.get("KV2_DEBUG") == "1"
    if dbg:
        dbg_h = nc.dram_tensor("dbg_h", [128, CHUNKS * 128], f16,
                               kind="ExternalOutput")
        dbg_tab = nc.dram_tensor("dbg_tab", [NC * QSIZE[0], 128], f16,
                                 kind="ExternalOutput")
        dbg_x1 = nc.dram_tensor("dbg_x1", [128, N_OWN], f16,
                                kind="ExternalOutput")
        dbg_pacc = nc.dram_tensor("dbg_pacc", [128, N_OWN], f16,
                                  kind="ExternalOutput")

    # ---------------- internal DRAM ----------------
    hq = [nc.dram_tensor(f"hq{qq}", [QSIZE[qq], 128], f16) for qq in range(NQ)]
    tab = [nc.dram_tensor(f"tab{qq}", [NC * QSIZE[qq], 128], f16,
                          addr_space="Shared") for qq in range(NQ)]

    # per-quarter chunk touch maps (static, identical on all cores)
    chw = [512] * (NCH - 1) + [N_OWN - 512 * (NCH - 1)]  # last chunk 256
    first_t = []   # per q: {chunk: first tile}
    last_t = []    # per q: {chunk: last tile}
    for qq in range(NQ):
        ft, lt = {}, {}
        for t, (a, b) in enumerate(plan[qq]):
            if b <= a:
                continue
            for k in range(a // 512, (b - 1) // 512 + 1):
                if k not in ft:
                    ft[k] = t
                lt[k] = t
        first_t.append(ft)
        last_t.append(lt)

    with tile.TileContext(nc) as tc:
        with (
            tc.tile_pool(name="const", bufs=1) as cp,
            tc.tile_pool(name="msgp", bufs=32) as msgp,
            tc.tile_pool(name="indp", bufs=32) as indp,
            tc.tile_pool(name="diagp", bufs=12) as diagp,
            tc.tile_pool(name="evp", bufs=8) as evp,
            tc.tile_pool(name="psum_h", bufs=2, space="PSUM") as pp_h,
            tc.tile_pool(name="psum_agg", bufs=4, space="PSUM") as pp_agg,
            tc.tile_pool(name="psum_t", bufs=2, space="PSUM") as pp_t,
        ):
            nc.gpsimd.load_library(library_config.mlp)
            nregs = {CALL: nc.gpsimd.to_reg(CALL)}
            for qq in range(NQ):
                tn = T_q[qq] * 128 - (T_q[qq] * 128 // CALL) * CALL
                if tn and tn not in nregs:
                    nregs[tn] = nc.gpsimd.to_reg(tn)

            # persistent SBUF
            xT = cp.tile([128, N_OWN], f16)
            nc.sync.dma_start(out=xT[:], in_=xT_in[:])
            wt = []
            for l in range(N_LAYERS):
                w = cp.tile([128, 128], f16, tag=f"w{l}")
                nc.sync.dma_start(out=w[:], in_=w_in[l][:])
                wt.append(w)
            dinv_sb = cp.tile([128, CHUNKS], f32)
            nc.sync.dma_start(out=dinv_sb[:], in_=dinv_in[:])
            pidx = cp.tile([128, 1], f32)
            nc.sync.dma_start(out=pidx[:], in_=pidx_in[:])
            iota = cp.tile([128, 2048], f16)
            nc.sync.dma_start(out=iota[:], in_=iota_in[:])
            gidx = cp.tile([128, t_tot * 8], i16)
            nc.sync.dma_start(out=gidx[:], in_=gidx_in[:])
            dstrel = cp.tile([128, t_tot], f32)
            nc.sync.dma_start(out=dstrel[:], in_=dstrel_in[:])
            sdw = cp.tile([128, t_tot], f32)
            nc.sync.dma_start(out=sdw[:], in_=sdw_in[:])
            ident = cp.tile([128, 128], f16)
            nc.sync.dma_start(out=ident[:], in_=ident_in[:])
            h_sb = cp.tile([128, CHUNKS, 128], f16)
            pacc = cp.tile([128, N_OWN], f16)
            b_sb = []
            if has_bias:
                for l in range(N_LAYERS):
                    bt = cp.tile([128, 1], f32, tag=f"b{l}")
                    nc.sync.dma_start(out=bt[:], in_=b_in[l][:])
                    b_sb.append(bt)

            # ---------------- temporal gate ----------------
            wg1 = cp.tile([128, 1], f32)
            nc.sync.dma_start(out=wg1[:], in_=wg1_in[:])
            bg1 = cp.tile([128, 1], f32)
            nc.sync.dma_start(out=bg1[:], in_=bg1_in[:])
            wg2 = cp.tile([128, 128], f32)
            nc.sync.dma_start(out=wg2[:], in_=wg2_in[:])
            bg2 = cp.tile([128, 1], f32)
            nc.sync.dma_start(out=bg2[:], in_=bg2_in[:])
            tsr = cp.tile([128, 1], f32)
            nc.sync.dma_start(out=tsr[:], in_=ts_in[:])

            tmp1 = cp.tile([128, 1], f32, tag="g1")
            nc.vector.tensor_mul(tmp1[:], wg1[:], tsr[:])
            tanh1 = cp.tile([128, 1], f32, tag="g2")
            nc.scalar.activation(
                tanh1[:], tmp1[:], mybir.ActivationFunctionType.Tanh, bias=bg1[:])
            ps_g = pp_h.tile([128, 1], f32, tag="psh")
            nc.tensor.matmul(ps_g[:], lhsT=wg2[:], rhs=tanh1[:], start=True, stop=True)
            gate_col = cp.tile([128, 1], f32, tag="gcol")
            nc.scalar.activation(
                gate_col[:], ps_g[:], mybir.ActivationFunctionType.Sigmoid,
                bias=bg2[:])

            # ---------------- layers ----------------
            for l in range(N_LAYERS):
                last = l == N_LAYERS - 1
                # h' = dinv * (x @ W) into h_sb; DMA per quarter + AllGather
                for c in range(CHUNKS):
                    ps_h = pp_h.tile([128, 128], f32, tag="psh")
                    nc.tensor.matmul(
                        ps_h[:], lhsT=xT[:, c * 128:(c + 1) * 128],
                        rhs=wt[l][:], start=True, stop=True)
                    nc.scalar.activation(
                        h_sb[:, c, :], ps_h[:],
                        mybir.ActivationFunctionType.Copy,
                        scale=dinv_sb[:, c:c + 1])
                    for qq in range(NQ):
                        if c == QCHUNK[qq + 1] - 1:
                            ca, cb = QCHUNK[qq], QCHUNK[qq + 1]
                            nc.sync.dma_start(
                                out=hq[qq][:].rearrange(
                                    "(c p) f -> p c f", p=128),
                                in_=h_sb[:, ca:cb, :])
                            nc.gpsimd.collective_compute(
                                "AllGather", mybir.AluOpType.bypass,
                                replica_groups=[list(range(NC))],
                                ins=[hq[qq][:]],
                                outs=[tab[qq][:]],
                            )

                if dbg and l == 0:
                    nc.scalar.dma_start(
                        out=dbg_h[:], in_=h_sb[:].rearrange("p c f -> p (c f)"))
                    nc.scalar.dma_start(out=dbg_tab[:], in_=tab[0][:])

                # ---- edge rounds, one per src quarter ----
                for r in range(NQ):
                    live = {}
                    n_calls = (T * 128 + CALL - 1) // CALL
                    msg_tiles = []
                    for t in range(T):
                        if t % (CALL // 128) == 0:
                            k = t // (CALL // 128)
                            n_idx = min(CALL, T * 128 - k * CALL)
                            msg = msgp.tile([128, CALL // 128, 128], f16,
                                            tag="msg")
                            gcol0 = (r * T * 128 + k * CALL) // 16
                            nc.gpsimd.dma_gather(
                                out_ap=msg[:, :n_idx // 128, :],
                                in_ap=tab[r][:],
                                idxs_ap=gidx[:, gcol0:gcol0 + n_idx // 16],
                                num_idxs=n_idx,
                                num_idxs_reg=call_reg if n_idx == CALL
                                else tail_reg,
                                elem_size=128,
                                queue_num=(r * n_calls + k) % 4)
                            msg_tiles.append(msg)
                            if dbg and l == 0 and r == 0 and k == 0:
                                nc.scalar.dma_start(
                                    out=dbg_msg[:],
                                    in_=msg[:].rearrange("p c f -> p (c f)"))
                        # init / reload psum chunks first touched here
                        for k512 in (bc_t[t], bc_t[t] + 1):
                            if k512 in live or k512 >= NCH:
                                continue
                            if k512 == bc_t[t] + 1 and w0_t[t] == 0:
                                continue
                            if first_t[k512] != t:
                                continue
                            ps = pp_agg.tile([128, 512], f32, tag="agg")
                            live[k512] = ps
                            wk = chw[k512]
                            if r == 0:
                                for j in range(-(-wk // 128)):
                                    cc = k512 * 4 + j
                                    dg = diagp.tile([128, 128], f16, tag="dg")
                                    nc.vector.tensor_scalar(
                                        out=dg[:], in0=iota[:, 0:128],
                                        scalar1=pidx[:],
                                        scalar2=dinv_sb[:, cc:cc + 1],
                                        op0=mybir.AluOpType.is_equal,
                                        op1=mybir.AluOpType.mult)
                                    nc.tensor.matmul(
                                        ps[:, j * 128:(j + 1) * 128],
                                        lhsT=h_sb[:, cc, :], rhs=dg[:],
                                        start=j == 0, stop=False)
                            else:
                                nc.tensor.matmul(
                                    ps[:, 0:wk], lhsT=ident[:],
                                    rhs=pacc[:, k512 * 512:k512 * 512 + wk],
                                    start=True, stop=False)
                        # indicator + segment matmuls
                        ind = indp.tile([128, 512], f16, tag="ind")
                        gt = r * T + t
                        nc.vector.tensor_scalar(
                            out=ind[:], in0=iota[:, w0_t[t]:w0_t[t] + 512],
                            scalar1=dstrel[:, gt:gt + 1],
                            scalar2=sdw[:, gt:gt + 1],
                            op0=mybir.AluOpType.is_equal,
                            op1=mybir.AluOpType.mult)
                        if dbg and l == 0 and r == 0 and t == 0:
                            nc.scalar.dma_start(out=dbg_ind[:], in_=ind[:])
                        msg = msg_tiles[t // (CALL // 128)]
                        mj = t % (CALL // 128)
                        w0 = w0_t[t]
                        bc = bc_t[t]
                        wA = min(512, chw[bc]) - w0
                        nc.tensor.matmul(
                            live[bc][:, w0:w0 + wA],
                            lhsT=msg[:, mj, :], rhs=ind[:, 0:wA],
                            start=False, stop=last_t[bc] == t)
                        if w0 > 0 and bc + 1 < NCH:
                            wB = min(w0, chw[bc + 1])
                            nc.tensor.matmul(
                                live[bc + 1][:, 0:wB],
                                lhsT=msg[:, mj, :],
                                rhs=ind[:, 512 - w0:512 - w0 + wB],
                                start=False, stop=last_t[bc + 1] == t)
                        # evict finished chunks
                        for k512 in [k for k, lt in last_t.items()
                                     if lt == t and k in live]:
                            ps = live.pop(k512)
                            wk = chw[k512]
                            if r < NQ - 1:
                                nc.scalar.activation(
                                    pacc[:, k512 * 512:k512 * 512 + wk],
                                    ps[:, 0:wk],
                                    mybir.ActivationFunctionType.Copy)
                            else:
                                xf = evp.tile([128, 512], f16, tag="xf")
                                nc.scalar.activation(
                                    xf[:, 0:wk], ps[:, 0:wk],
                                    mybir.ActivationFunctionType.Relu,
                                    bias=b_sb[l][:] if has_bias else 0.0)
                                if not last:
                                    nc.vector.tensor_scalar_mul(
                                        xT[:, k512 * 512:k512 * 512 + wk],
                                        xf[:, 0:wk], gate_col[:])
                                else:
                                    xg = evp.tile([128, 512], f16, tag="xg")
                                    nc.vector.tensor_scalar_mul(
                                        xg[:, 0:wk], xf[:, 0:wk], gate_col[:])
                                    for j in range(-(-wk // 128)):
                                        cc = k512 * 4 + j
                                        ps_t = pp_t.tile(
                                            [128, 128], f32, tag="pst")
                                        nc.tensor.matmul(
                                            ps_t[:],
                                            lhsT=xg[:, j * 128:(j + 1) * 128],
                                            rhs=ident[:],
                                            start=True, stop=True)
                                        rowo = evp.tile(
                                            [128, 128], f32, tag="rowo")
                                        nc.scalar.activation(
                                            rowo[:], ps_t[:],
                                            mybir.ActivationFunctionType.Copy)
                                        nc.sync.dma_start(
                                            out=out_ext[
                                                cc * 128:(cc + 1) * 128, :],
                                            in_=rowo[:])
                    assert not live
                    if dbg and l == 0 and r == 0:
                        nc.scalar.dma_start(out=dbg_pacc[:], in_=pacc[:])
                if dbg and l == 0:
                    nc.scalar.dma_start(out=dbg_x1[:], in_=xT[:])

    mybir.codegen_inst_isa_subclasses(nc)


_CACHE = {}


def _get_program(plan, has_bias):
    key = (plan, has_bias, os.environ.get("KV2_DEBUG"))
    if key not in _CACHE:
        nc = bass.Bass(num_devices=NC, num_swdge_queues=4)
        _build(nc, plan, has_bias)
        _CACHE[key] = nc
    return _CACHE[key]


def _prepare(inputs):
    x = np.asarray(inputs["x"], np.float32)
    edge_index = np.asarray(inputs["edge_index"])
    ts = np.asarray(inputs["timestamp"], np.float32).reshape(-1)[0]
    Ws = [np.asarray(inputs[f"W{l}"], np.float32) for l in range(N_LAYERS)]
    bs = [np.asarray(inputs[f"b{l}"], np.float32) for l in range(N_LAYERS)]
    Wg1 = np.asarray(inputs["Wg1"], np.float32)
    bg1 = np.asarray(inputs["bg1"], np.float32)
    Wg2 = np.asarray(inputs["Wg2"], np.float32)
    bg2 = np.asarray(inputs["bg2"], np.float32)

    dinv, gidx, dstrel, sdw, plan = _prep_graph(edge_index)
    has_bias = any(np.abs(b).max() > 0 for b in bs)

    ident = np.eye(128, dtype=np.float16)
    iota = np.tile(np.arange(2048, dtype=np.float16), (128, 1))
    pidx = np.arange(128, dtype=np.float32).reshape(128, 1)
    in_maps = []
    for c in range(NC):
        lo = c * N_OWN
        hi = min((c + 1) * N_OWN, N_NODES)
        xb = np.zeros((N_OWN, 128), np.float16)
        xb[: hi - lo] = x[lo:hi].astype(np.float16)
        dv = np.ones(N_OWN, np.float32)
        dv[: hi - lo] = dinv[lo:hi]
        m = {
            "xT_in": np.ascontiguousarray(xb.T),
            "dinv_in": np.ascontiguousarray(dv.reshape(CHUNKS, 128).T),
            "pidx_in": pidx,
            "iota_in": iota,
            "gidx_in": _wrap_idx(gidx[c]),
            "dstrel_in": np.ascontiguousarray(
                dstrel[c].reshape(-1, 128).T),
            "sdw_in": np.ascontiguousarray(
                sdw[c].reshape(-1, 128).T),
            "ident_in": ident,
            "wg1_in": Wg1.reshape(128, 1),
            "bg1_in": bg1.reshape(128, 1),
            "wg2_in": np.ascontiguousarray(Wg2),
            "bg2_in": bg2.reshape(128, 1),
            "ts_in": np.full((128, 1), ts, np.float32),
        }
        for l in range(N_LAYERS):
            m[f"w{l}"] = Ws[l].astype(np.float16)
            if has_bias:
                m[f"b{l}"] = bs[l].astype(np.float32).reshape(128, 1)
        in_maps.append(m)
    return in_maps, plan, has_bias


def _run(inputs, trace=False):
    in_maps, plan, has_bias = _prepare(inputs)
    nc = _get_program(plan, has_bias)
    last_err = None
    for attempt in range(3):
        try:
            res = run_bass_kernel_spmd(
                nc, in_maps, core_ids=list(range(NC)), trace=trace)
            break
        except Exception as e:  # wedged device recovers on retry
            last_err = e
            if attempt == 2:
                raise
    blocks = [res.results[c]["out_ext"] for c in range(NC)]
    out = np.concatenate(blocks, axis=0)[:N_NODES].astype(np.float32)
    return out, res


def kernel(**inputs) -> np.ndarray:
    out, _ = _run(inputs, trace=False)
    return out


def kernel_traced(**inputs):
    return _run(inputs, trace=True)
